# revision 12
# baseline (speedup 1.0000x reference)
"""AttentionPool2d (masked, 100-mask sparse attention) on 8 TRN2 NeuronCores.

Algorithm notes
---------------
The reference returns out[0] - only the cls/mean query token. So per (b, h)
we only need scores0[m] = q0 . k[m], the 100-mask softmax over keys, the sum
over masks, and one weighted sum over v. Per-core sharding is by head:
core c owns heads {2c, 2c+1} = E-channels [128c, 128c+128). q/k/v weight
rows and c_w columns are sharded accordingly (weights fully partitioned,
no replication); x / pos_emb / (subsampled) mask are replicated.

Perf design (v3):
- all large inputs are host-packed to bf16 (halves HBM traffic; rel-err
  budget is 2e-2, measured ~4e-3).
- sigmoid(x) == 0.5*tanh(x/2)+0.5 so every ACT op (tanh, exp) lives in the
  single `exp_and_others` table set; a dummy exp preloads it once.
- one dma_start per logical input (issue costs ~0.65us on the issuing
  queue), split across the two HWDGE queues (sync + scalar), ordered so x
  (the critical path head) lands first.
- q0 projection matmuls issue before K/V per e-tile so the softmax chains
  are not gated on the q0 scalar path.
- XS assembly is 2 big DVE adds + a strided mean-column fix per batch.
- final cross-core reduction: AllGather of the per-core c-proj partial
  [128, 16] (o-major) + local tree-sum + bias (AG mesh ~5us vs AR ~18us).
  A tiny dummy AllGather issues at t~0 to absorb the ~11us ncfw wake /
  inter-core sync cost before the real one.
- output is returned o-major [128, 2*8] and unpacked to [B, O] on host.

The token axis is padded 197 -> 198 per block (pad cols are zero in x/pos
so K/V pad cols are bias-only; mask pad col is zeroed and the exp row-sum
gets a "-1" correction; the w-pad col is excluded from the v-reduction).
"""
import os

import numpy as np

B = 2
H = 16
E = 1024
SP = 14
S = SP * SP          # 196
NM = 100
L = S + 1            # 197
LP = L + 1           # 198 padded
HD = 64
NET = 8              # e-tiles of 128
NCORES = 8
SCALE = HD ** -0.5   # 0.125

_STATE = {}


def _build():
    import concourse.bass as bass
    import concourse.mybir as mybir
    from concourse import bacc, tile

    F32 = mybir.dt.float32
    BF16 = mybir.dt.bfloat16
    AF = mybir.ActivationFunctionType
    ALU = mybir.AluOpType

    nc = bacc.Bacc("TRN2", target_bir_lowering=False, debug=False,
                   num_devices=NCORES)

    x_ap = nc.dram_tensor("x", [B, 128, NET * LP], BF16, kind="ExternalInput").ap()
    pos_ap = nc.dram_tensor("pos_t", [128, NET * LP], BF16, kind="ExternalInput").ap()
    qkvw_ap = nc.dram_tensor("qkvw", [128, NET * 384], BF16, kind="ExternalInput").ap()
    qkvb_ap = nc.dram_tensor("qkvb", [128, 3], F32, kind="ExternalInput").ap()
    cwt_ap = nc.dram_tensor("cwt", [128, E], BF16, kind="ExternalInput").ap()
    cbt_ap = nc.dram_tensor("cbt", [128, 2 * NET], F32, kind="ExternalInput").ap()
    mask_ap = nc.dram_tensor("mask", [NM, B * S], BF16, kind="ExternalInput").ap()
    out_ap = nc.dram_tensor("out", [128, 2 * NET], F32, kind="ExternalOutput").ap()

    with tile.TileContext(nc) as tc:
        with (
            tc.tile_pool(name="sb", bufs=1) as sb,
            tc.tile_pool(name="sb4", bufs=4) as sb4,
            tc.tile_pool(name="ps_kv", bufs=1, space="PSUM") as ps_kv,
            tc.tile_pool(name="ps_sw", bufs=4, space="PSUM") as ps_sw,
            tc.tile_pool(name="ps_small", bufs=1, space="PSUM") as ps_small,
            tc.tile_pool(name="dram", bufs=1, space="DRAM") as dram,
        ):
            # ---- input DMAs (sync queue: x first, then weights) ----
            X = []
            for b in range(B):
                xb = sb.tile([128, NET * LP], BF16, tag=f"x{b}")
                nc.sync.dma_start(xb[:], x_ap[b])
                X.append(xb)
            QKVW = sb.tile([128, NET * 384], BF16, tag="qkvw")
            nc.sync.dma_start(QKVW[:], qkvw_ap[:])
            QKVB = sb.tile([128, 3], F32, tag="qkvb")
            nc.sync.dma_start(QKVB[:], qkvb_ap[:])

            # ---- input DMAs (scalar queue) + ACT table preload ----
            PT = sb.tile([128, NET * LP], BF16, tag="pt")
            nc.gpsimd.dma_start(PT[:], pos_ap[:])
            MIN = sb.tile([NM, B * S], BF16, tag="min")
            nc.gpsimd.dma_start(MIN[:], mask_ap[:])
            dummy = sb.tile([1, 2], F32, tag="dummy")
            nc.vector.memset(dummy[:], 0.0)
            nc.scalar.activation(dummy[:], dummy[:], AF.Exp)
            CWT = sb.tile([128, E], BF16, tag="cwt")
            nc.gpsimd.dma_start(CWT[:], cwt_ap[:])
            CBT = sb.tile([128, 2 * NET], F32, tag="cbt")
            nc.gpsimd.dma_start(CBT[:], cbt_ap[:])

            # ---- dummy collective to warm the CC path / sync cores ----
            dc_sb = sb.tile([1, 8], F32, tag="dc_sb")
            nc.vector.memset(dc_sb[:], 0.0)
            dc_in = dram.tile([1, 8], F32)
            nc.gpsimd.dma_start(dc_in[:], dc_sb[:])
            dc_out = dram.tile([NCORES, 8], F32)
            nc.gpsimd.collective_compute(
                "AllGather", mybir.AluOpType.bypass,
                replica_groups=[list(range(NCORES))],
                ins=[dc_in.opt()], outs=[dc_out.opt()])

            # ---- masks: sigmoid via 0.5*tanh(x/2)+0.5 (same set as exp) ----
            mt = sb.tile([NM, B * S], F32, tag="mt")
            nc.scalar.activation(mt[:], MIN[:], AF.Tanh, scale=0.5)
            M_sb = sb.tile([NM, B * LP], F32, tag="msb")
            MV = M_sb[:].rearrange("n (b l) -> n b l", b=B)
            for b in range(B):
                nc.vector.tensor_scalar(
                    MV[:, b, 1:L], mt[:, b * S:(b + 1) * S], 0.5, 0.5,
                    ALU.mult, ALU.add)
            nc.vector.memset(M_sb[:, 0:B * LP:LP], 1.0)
            nc.vector.memset(M_sb[:, L:B * LP:LP], 0.0)

            # ---- XS assembly: XS = x + pos; col0 = mean(x)+pos0 per block --
            XS = [sb.tile([128, NET * LP], BF16, tag=f"xs{b}", name=f"xs{b}")
                  for b in range(B)]
            for b in range(B):
                nc.vector.tensor_add(XS[b][:], X[b][:], PT[:])
                msum = sb.tile([128, NET], F32, tag=f"msum{b}")
                nc.vector.reduce_sum(
                    msum[:], X[b][:].rearrange("p (n c) -> p n c", c=LP),
                    axis=mybir.AxisListType.X)
                msd = sb.tile([128, NET], BF16, tag=f"msd{b}")
                nc.vector.tensor_scalar_mul(msd[:], msum[:], 1.0 / S)
                # mean+pos0 into XS col0 of each block
                nc.vector.tensor_add(XS[b][:, 0::LP], msd[:], PT[:, 0::LP])

            # ---- q0/K/V projections (bf16) ----
            # two PSUM banks: bankK = [K0 | K1], bankV = [V0 | V1 | q0b0 |
            # q0b1]. Accumulation groups sharing a bank are issued
            # contiguously so groups on one tile stay sequential
            # (interleaving deadlocks the tile scheduler's group tracking).
            # q0 rhs = XS cols {0, 197} per block (col 197 is zero padding).
            bankK = ps_kv.tile([128, 2 * LP], F32, tag="bankK", name="bankK")
            bankV = ps_kv.tile([128, 2 * LP + 4], F32, tag="bankV",
                               name="bankV")
            K_ps = [bankK[:, b * LP:(b + 1) * LP] for b in range(B)]
            V_ps = [bankV[:, b * LP:(b + 1) * LP] for b in range(B)]
            q0_ps = [bankV[:, 2 * LP + 2 * b: 2 * LP + 2 * b + 2]
                     for b in range(B)]

            def group(out, col0, rhs_of_et):
                for et in range(NET):
                    nc.tensor.matmul(out, QKVW[:, et * 384 + col0:
                                               et * 384 + col0 + 128],
                                     rhs_of_et(et),
                                     start=(et == 0), stop=(et == NET - 1))

            group(K_ps[0], 0, lambda et: XS[0][:, et * LP:(et + 1) * LP])
            group(q0_ps[0], 256,
                  lambda et: XS[0][:, et * LP: (et + 1) * LP: LP - 1])
            group(q0_ps[1], 256,
                  lambda et: XS[1][:, et * LP: (et + 1) * LP: LP - 1])
            group(K_ps[1], 0, lambda et: XS[1][:, et * LP:(et + 1) * LP])
            group(V_ps[0], 128, lambda et: XS[0][:, et * LP:(et + 1) * LP])
            group(V_ps[1], 128, lambda et: XS[1][:, et * LP:(et + 1) * LP])

            # q0 scaled+biased: (q0_raw + q_b) * 0.125, replicated over 100
            # mask-partitions per head slice.
            q0v = sb.tile([128, B], F32, tag="q0v")
            nc.vector.tensor_scalar(q0v[:], bankV[:, 2 * LP:2 * LP + 4:2],
                                    QKVB[:, 2:3], SCALE, ALU.add, ALU.mult)
            ones_q = sb.tile([128, NM], F32, tag="ones_q")
            nc.vector.memset(ones_q[:], 1.0)
            ones_r = sb.tile([NM, HD], F32, tag="ones_r")
            nc.vector.memset(ones_r[:], 1.0)
            Q0R = []
            for b in range(B):
                q0r = sb.tile([128, NM], BF16, tag=f"q0r{b}")
                for h in range(2):
                    sl = slice(h * HD, (h + 1) * HD)
                    nc.vector.tensor_scalar_mul(q0r[sl, :], ones_q[sl, :],
                                                q0v[sl, b:b + 1])
                Q0R.append(q0r)

            K_sb, V_sb = [], []
            for b in range(B):
                k_sb = sb.tile([128, LP], BF16, tag=f"k_sb{b}")
                nc.vector.tensor_scalar_add(k_sb[:], K_ps[b], QKVB[:, 0:1])
                K_sb.append(k_sb)
                v_sb = sb.tile([128, LP], F32, tag=f"v_sb{b}")
                nc.vector.tensor_scalar_add(v_sb[:], V_ps[b], QKVB[:, 1:2])
                V_sb.append(v_sb)

            # ---- per (b, h): scores -> masked softmax -> attn ----
            A0 = sb.tile([128, B], F32, tag="a0")
            for b in range(B):
                for h in range(2):
                    sl = slice(h * HD, (h + 1) * HD)
                    s_ps = ps_sw.tile([NM, LP], F32, tag="sw")
                    nc.tensor.matmul(s_ps[:], Q0R[b][sl, :], K_sb[b][sl, :],
                                     start=True, stop=True)
                    sm = sb4.tile([NM, LP], F32, tag="sm")
                    nc.vector.tensor_mul(sm[:], s_ps[:],
                                         M_sb[:, b * LP:(b + 1) * LP])
                    e_sb = sb4.tile([NM, LP], BF16, tag="e")
                    rs = sb4.tile([NM, 1], F32, tag="rs")
                    nc.scalar.activation(e_sb[:], sm[:], AF.Exp,
                                         accum_out=rs[:])
                    # pad col of sm is 0 -> exp=1; subtract from row sum
                    rcol = sb4.tile([NM, 1], F32, tag="rc")
                    nc.vector.tensor_scalar_add(rcol[:], rs[:], -1.0)
                    nc.vector.reciprocal(rcol[:], rcol[:])
                    rrep = sb4.tile([NM, HD], BF16, tag="rrep")
                    nc.vector.tensor_scalar_mul(rrep[:], ones_r[:], rcol[:])
                    w_ps = ps_sw.tile([HD, LP], F32, tag="sw")
                    nc.tensor.matmul(w_ps[:], rrep[:], e_sb[:],
                                     start=True, stop=True)
                    # w pad col = sum_n r_n != 0 and V pad col = vb: exclude
                    # the pad col from the weighted-v reduction.
                    t_mul = sb4.tile([HD, L], F32, tag="t_mul")
                    nc.vector.tensor_mul(t_mul[:], w_ps[:, 0:L],
                                         V_sb[b][sl, 0:L])
                    nc.vector.reduce_sum(A0[sl, b:b + 1], t_mul[:],
                                         axis=mybir.AxisListType.X)

            # ---- c-proj transposed: part[p, 2*ot+b] = sum_ch A0 * c_w ----
            A0r = sb.tile([128, B], BF16, tag="a0r")
            nc.vector.tensor_scalar_add(A0r[:], A0[:], 0.0)
            o_ps = ps_small.tile([128, 2 * NET], F32, tag="ops")
            for j in range(NET):
                nc.tensor.matmul(o_ps[:, 2 * j: 2 * j + 2],
                                 CWT[:, j * 128:(j + 1) * 128], A0r[:],
                                 start=True, stop=True)
            part_sb = sb.tile([128, 2 * NET], F32, tag="part_sb")
            nc.vector.tensor_copy(part_sb[:], o_ps[:])

            # ---- AllGather partials + local tree-sum + bias ----
            part = dram.tile([128, 2 * NET], F32)
            nc.gpsimd.dma_start(part[:], part_sb[:])
            red = dram.tile([NCORES * 128, 2 * NET], F32)
            nc.gpsimd.collective_compute(
                "AllGather", mybir.AluOpType.bypass,
                replica_groups=[list(range(NCORES))],
                ins=[part.opt()], outs=[red.opt()])
            G = 2 * NET
            red_sb = sb.tile([128, NCORES * G], F32, tag="red_sb")
            nc.sync.dma_start(
                red_sb[:].rearrange("p (r c) -> p r c", r=NCORES),
                red[:].rearrange("(r p) c -> p r c", p=128))
            t4 = sb.tile([128, 4 * G], F32, tag="t4")
            for j in range(4):
                nc.vector.tensor_add(t4[:, j * G:(j + 1) * G],
                                     red_sb[:, 2 * j * G:(2 * j + 1) * G],
                                     red_sb[:, (2 * j + 1) * G:(2 * j + 2) * G])
            t2 = sb.tile([128, 2 * G], F32, tag="t2")
            for j in range(2):
                nc.vector.tensor_add(t2[:, j * G:(j + 1) * G],
                                     t4[:, 2 * j * G:(2 * j + 1) * G],
                                     t4[:, (2 * j + 1) * G:(2 * j + 2) * G])
            t1 = sb.tile([128, G], F32, tag="t1")
            nc.vector.tensor_add(t1[:], t2[:, 0:G], t2[:, G:2 * G])
            out_sb = sb.tile([128, G], F32, tag="out_sb")
            nc.vector.tensor_add(out_sb[:], t1[:], CBT[:])
            nc.sync.dma_start(out_ap[:], out_sb[:])

    nc.compile()
    return nc


def _get_nc():
    if "nc" not in _STATE:
        _STATE["nc"] = _build()
    return _STATE["nc"]


def _pack_blocks(a, block_in, pad_to, col_off):
    """[rows=8*128, cols=block_in] -> bf16 [128, 8*pad_to], zero elsewhere.

    Block et occupies cols [et*pad_to + col_off, et*pad_to + col_off + block_in).
    """
    a = np.asarray(a, dtype=np.float32)
    t = a.reshape(NET, 128, block_in).transpose(1, 0, 2)  # [128, 8, block_in]
    out = np.zeros((128, NET, pad_to), np.float32)
    out[:, :, col_off:col_off + block_in] = t
    return np.ascontiguousarray(
        out.reshape(128, NET * pad_to)).astype(_bf16())


def _bf16():
    import ml_dtypes
    return ml_dtypes.bfloat16


def host_inputs(inputs):
    x = np.asarray(inputs["x"], np.float32)
    mask_feature = np.asarray(inputs["mask_feature"], np.float32)
    pos_emb = np.asarray(inputs["pos_emb"], np.float32)
    q_w = np.asarray(inputs["q_w"], np.float32)
    q_b = np.asarray(inputs["q_b"], np.float32)
    k_w = np.asarray(inputs["k_w"], np.float32)
    k_b = np.asarray(inputs["k_b"], np.float32)
    v_w = np.asarray(inputs["v_w"], np.float32)
    v_b = np.asarray(inputs["v_b"], np.float32)
    c_w = np.asarray(inputs["c_w"], np.float32)
    c_b = np.asarray(inputs["c_b"], np.float32)

    # replicated tensors (packed layouts, pure data movement + dtype cast)
    x_flat = x.reshape(B, E, S)
    # x block: [0 | x tokens (196) | 0]; pos block: [pos0..pos196 | 0]
    x_packed = np.stack([_pack_blocks(x_flat[b], S, LP, 1) for b in range(B)])
    pos_packed = _pack_blocks(np.ascontiguousarray(pos_emb.T), L, LP, 0)
    # mask[n, b*S + s] = mask_feature[b, n, ::8, ::8]
    mask12 = np.ascontiguousarray(
        mask_feature[:, :, ::8, ::8].reshape(B, NM, S).transpose(1, 0, 2)
        .reshape(NM, B * S)).astype(_bf16())
    # cbt[p, 2*ot+b] = c_b[ot*128+p]
    cbt = np.ascontiguousarray(
        np.repeat(c_b.reshape(NET, 128).T[:, :, None], B, axis=2
                  ).reshape(128, NET * B))

    in_maps = []
    for c in range(NCORES):
        ch = slice(c * 128, (c + 1) * 128)
        qkvw = np.concatenate(
            [k_w[ch].T, v_w[ch].T, q_w[ch].T], axis=1)  # [1024, 384]
        in_maps.append({
            "x": x_packed,
            "pos_t": pos_packed,
            "qkvw": _pack_blocks(qkvw, 384, 384, 0),
            "qkvb": np.ascontiguousarray(
                np.stack([k_b[ch], v_b[ch], q_b[ch]], axis=1)),
            "cwt": np.ascontiguousarray(c_w[:, ch].T).astype(_bf16()),
            "cbt": cbt,
            "mask": mask12,
        })
    return in_maps


def unpack_out(out):
    # out[p, 2*ot+b] -> [B, O]
    o = np.asarray(out, np.float32).reshape(128, NET, B)
    return np.ascontiguousarray(o.transpose(2, 1, 0).reshape(B, E))


def kernel(**inputs):
    in_maps = host_inputs(inputs)

    from concourse.bass_utils import run_bass_kernel_spmd

    nc = _get_nc()
    trace = bool(int(os.environ.get("KERNEL_TRACE", "0")))
    if trace:
        try:
            import ntff_hook
            ntff_hook.install()
        except Exception:
            pass
    res = run_bass_kernel_spmd(nc, in_maps, list(range(NCORES)), trace=trace)
    _STATE["last_exec_ns"] = res.exec_time_ns
    _STATE["last_results"] = res
    return unpack_out(res.results[0]["out"])
